# revision 11
# baseline (speedup 1.0000x reference)
"""Trainium2 Bass kernel for the nn_MultiHeadAttention problem.

Data-parallel over batch: each of the 8 NeuronCores processes one batch
element independently (no collectives).

Per-core dataflow (L=1024, E=1024, H=16, D=64; q/k tiles of 128,
e-chunks of 128 = 2 heads):

  host:  QT/KT/VT = Q[b].T etc in fp16/bf16 (no on-chip transposes),
         W2[c] = blockdiag(HeadLinear[2c], HeadLinear[2c+1]),
         masks as f32 [128, 8] in tile layout, O in bf16.
  proj:  qT2/kT2 [128,1024] fp16 = W2[c].T @ QT_chunk (both heads at once)
         v2 [128k,130] bf16 per k-tile = (VT_chunk.T @ W2b[c]) * kmask,
         plus a kmask "ones" column per head.
  scores: s[k,q] psum = kT_h_slice.T @ qT_h (fp16); P = exp(s/8) via one
         ACT op per [128,1024] tile, output bf16.  No max subtraction
         (|s|/8 <~ 13), no explicit -1e10 mask: masked keys have v-rows
         and ones-column zeroed, which reproduces masked_fill+softmax
         exactly (exp(-1e10) == 0 in f32).
  PV:    out[65,q] psum = sum_k v2_slice.T @ P_slice (bf16, 512-wide
         moving operand); row 64 is the softmax denominator.  Normalize:
         reciprocal of row 64 -> bf16, broadcast to 64 partitions via a
         ones[1,64] matmul, one DVE multiply writes C^T rows directly.
  final: Y[q,:] psum = sum_c CT_slice.T @ O_chunk (bf16), multiplied by
         the query mask during psum->sbuf evacuation, DMA'd out.
"""

import os
import sys

import numpy as np

try:
    import concourse  # noqa: F401
except ImportError:  # pragma: no cover
    for _p in ("/opt/trn_rl_repo", os.path.expanduser("~/.axon_site/_ro/trn_rl_repo")):
        if os.path.isdir(_p) and _p not in sys.path:
            sys.path.insert(0, _p)

import ml_dtypes

import concourse.bass as bass
import concourse.tile as tile
from concourse import bacc, mybir

B, L, E, H, D = 8, 1024, 1024, 16, 64
P = 128          # partitions
NT = L // P      # 8 q/k tiles
NCH = E // P     # 8 e-chunks (2 heads each)
F32 = mybir.dt.float32
F16 = mybir.dt.float16
BF16 = mybir.dt.bfloat16


def build_bass():
    nc = bacc.Bacc(None, target_bir_lowering=False, debug=False)

    QT = nc.declare_dram_parameter("QT", [E, L], F16, isOutput=False)
    KT = nc.declare_dram_parameter("KT", [E, L], F16, isOutput=False)
    VT = nc.declare_dram_parameter("VT", [E, L], BF16, isOutput=False)
    W2 = nc.declare_dram_parameter("W2", [P, NCH, P], F16, isOutput=False)
    W2b = nc.declare_dram_parameter("W2b", [P, NCH, P], BF16, isOutput=False)
    OB = nc.declare_dram_parameter("OB", [E, E], BF16, isOutput=False)
    KM = nc.declare_dram_parameter("KM", [P, NT], F32, isOutput=False)
    QM = nc.declare_dram_parameter("QM", [P, NT], F32, isOutput=False)
    Y = nc.declare_dram_parameter("Y", [L, E], F32, isOutput=True)

    with tile.TileContext(nc) as tc:
        with (
            tc.tile_pool(name="singles", bufs=1) as singles,
            tc.tile_pool(name="qkT", bufs=2) as qkT,
            tc.tile_pool(name="vaug", bufs=2) as vaug,
            tc.tile_pool(name="ppool", bufs=2) as ppool,
            tc.tile_pool(name="ystage", bufs=2) as ystage,
            tc.tile_pool(name="rpool", bufs=3) as rpool,
            tc.tile_pool(name="bcpool", bufs=2) as bcpool,
            tc.tile_pool(name="psbig", bufs=2, space="PSUM") as psbig,
            tc.tile_pool(name="pspv", bufs=1, space="PSUM") as pspv,
            tc.tile_pool(name="pssmall", bufs=2, space="PSUM") as pssmall,
        ):
            # --- persistent SBUF tensors -------------------------------
            qts = singles.tile([P, NCH, L], F16)
            kts = singles.tile([P, NCH, L], F16)
            vts = singles.tile([P, NCH, L], BF16)
            obs = singles.tile([P, NCH, E], BF16)
            w2s = singles.tile([P, NCH, P], F16)
            w2bs = singles.tile([P, NCH, P], BF16)
            kms = singles.tile([P, NT], F32)
            qms = singles.tile([P, NT], F32)
            ct = singles.tile([P, NCH, L], BF16)

            # --- input DMAs (small/consts first, then per-chunk) -------
            nc.gpsimd.dma_start(out=w2s[:], in_=W2[:])
            nc.gpsimd.dma_start(out=w2bs[:], in_=W2b[:])
            nc.gpsimd.dma_start(out=kms[:], in_=KM[:])
            nc.gpsimd.dma_start(out=qms[:], in_=QM[:])
            for c in range(NCH):
                nc.gpsimd.dma_start(out=qts[:, c, :], in_=QT[c * P:(c + 1) * P, :])
                nc.gpsimd.dma_start(out=kts[:, c, :], in_=KT[c * P:(c + 1) * P, :])
                nc.gpsimd.dma_start(out=vts[:, c, :], in_=VT[c * P:(c + 1) * P, :])
            for c in range(NCH):
                nc.gpsimd.dma_start(out=obs[:, c, :], in_=OB[c * P:(c + 1) * P, :])

            # --- main loop over e-chunks (2 heads each) ----------------
            for c in range(NCH):
                # projections for both heads of this chunk
                qt2 = qkT.tile([P, L], F16, tag="qt2")
                kt2 = qkT.tile([P, L], F16, tag="kt2")
                for dst, src in ((qt2, qts), (kt2, kts)):
                    for hf in range(2):
                        ps = pssmall.tile([P, 512], F32, tag="small")
                        nc.tensor.matmul(
                            out=ps[:],
                            lhsT=w2s[:, c, :],
                            rhs=src[:, c, 512 * hf:512 * (hf + 1)],
                            start=True, stop=True,
                        )
                        nc.vector.tensor_copy(dst[:, 512 * hf:512 * (hf + 1)], ps[:])

                # v projection + key-mask + ones-column, both heads
                v2 = vaug.tile([P, NT, 130], BF16)
                for t in range(NT):
                    ps = pssmall.tile([P, P], F32, tag="small")
                    nc.tensor.matmul(
                        out=ps[:],
                        lhsT=vts[:, c, t * P:(t + 1) * P],
                        rhs=w2bs[:, c, :],
                        start=True, stop=True,
                    )
                    nc.vector.tensor_scalar(
                        out=v2[:, t, 0:64], in0=ps[:, 0:64],
                        scalar1=kms[:, t:t + 1], scalar2=None,
                        op0=mybir.AluOpType.mult,
                    )
                    nc.vector.tensor_scalar(
                        out=v2[:, t, 65:129], in0=ps[:, 64:128],
                        scalar1=kms[:, t:t + 1], scalar2=None,
                        op0=mybir.AluOpType.mult,
                    )
                # denominator "ones" columns = key mask itself
                nc.vector.tensor_copy(v2[:, :, 64], kms[:, :])
                nc.vector.tensor_copy(v2[:, :, 129], kms[:, :])

                for hf in range(2):
                    hq = qt2[64 * hf:64 * hf + 64, :]
                    hk = kt2[64 * hf:64 * hf + 64, :]
                    # scores (transposed, [k, q]) + exp -> P (bf16)
                    pt = ppool.tile([P, NT, L], BF16)
                    for t in range(NT):
                        sps = psbig.tile([P, L], F32, tag="big")
                        for qh in range(2):
                            nc.tensor.matmul(
                                out=sps[:, 512 * qh:512 * (qh + 1)],
                                lhsT=hk[:, t * P:(t + 1) * P],
                                rhs=hq[:, 512 * qh:512 * (qh + 1)],
                                start=True, stop=True,
                            )
                        nc.scalar.activation(
                            out=pt[:, t, :], in_=sps[:],
                            func=mybir.ActivationFunctionType.Exp,
                            scale=0.125,
                        )
                    # PV: out[65, q] accumulated over k-tiles, wide rhs
                    pv = pspv.tile([65, L], F32)
                    for kt in range(NT):
                        for qh in range(2):
                            nc.tensor.matmul(
                                out=pv[:, 512 * qh:512 * (qh + 1)],
                                lhsT=v2[:, kt, 65 * hf:65 * hf + 65],
                                rhs=pt[:, kt, 512 * qh:512 * (qh + 1)],
                                start=(kt == 0), stop=(kt == NT - 1),
                            )
                    # normalize: 1/denom -> broadcast to 64 partitions -> mult
                    rcp = rpool.tile([1, L], BF16)
                    with nc.allow_low_precision(reason="softmax recip in bf16"):
                        nc.vector.reciprocal(out=rcp[:], in_=pv[64:65, :])
                    bcs = bcpool.tile([64, L], BF16)
                    nc.gpsimd.partition_broadcast(out_ap=bcs[:], in_ap=rcp[:])
                    nc.vector.tensor_mul(
                        ct[64 * hf:64 * hf + 64, c, :], pv[0:64, :], bcs[:])

            # --- output projection ------------------------------------
            for t in range(NT):
                yps = psbig.tile([P, E], F32, tag="big")
                for c in range(NCH):
                    for eh in range(2):
                        nc.tensor.matmul(
                            out=yps[:, 512 * eh:512 * (eh + 1)],
                            lhsT=ct[:, c, t * P:(t + 1) * P],
                            rhs=obs[:, c, 512 * eh:512 * (eh + 1)],
                            start=(c == 0), stop=(c == NCH - 1),
                        )
                ys = ystage.tile([P, E], F32)
                nc.vector.tensor_scalar(
                    out=ys[:], in0=yps[:],
                    scalar1=qms[:, t:t + 1], scalar2=None,
                    op0=mybir.AluOpType.mult,
                )
                nc.gpsimd.dma_start(out=Y[t * P:(t + 1) * P, :], in_=ys[:])

    nc.compile()
    return nc


def make_core_inputs(Q, K, V, HeadLinear, OutputLiner, QMask, KMask):
    """Host-side sharding/layout prep. Returns list of per-core in_maps."""
    bf16 = ml_dtypes.bfloat16
    w2 = np.zeros((P, NCH, P), dtype=np.float32)
    hl = np.asarray(HeadLinear, dtype=np.float32)
    for c in range(NCH):
        w2[0:64, c, 0:64] = hl[2 * c]
        w2[64:128, c, 64:128] = hl[2 * c + 1]
    w2f = w2.astype(np.float16)
    w2b = w2.astype(bf16)
    ob = np.asarray(OutputLiner, dtype=np.float32).astype(bf16)

    in_maps = []
    for b in range(B):
        qt = np.asarray(Q[b], dtype=np.float32).T.astype(np.float16)
        kt = np.asarray(K[b], dtype=np.float32).T.astype(np.float16)
        vt = np.asarray(V[b], dtype=np.float32).T.astype(bf16)
        km = np.ascontiguousarray(
            np.asarray(KMask[b]).astype(np.float32).reshape(NT, P).T)
        qm = np.ascontiguousarray(
            np.asarray(QMask[b]).astype(np.float32).reshape(NT, P).T)
        in_maps.append({
            "QT": np.ascontiguousarray(qt), "KT": np.ascontiguousarray(kt),
            "VT": np.ascontiguousarray(vt),
            "W2": w2f, "W2b": w2b, "OB": ob,
            "KM": km, "QM": qm,
        })
    return in_maps


_NC_CACHE = None


def _get_nc():
    global _NC_CACHE
    if _NC_CACHE is None:
        _NC_CACHE = build_bass()
    return _NC_CACHE


def kernel(Q, K, V, HeadLinear, OutputLiner, QMask, KMask):
    from concourse.bass_utils import run_bass_kernel_spmd

    nc = _get_nc()
    in_maps = make_core_inputs(Q, K, V, HeadLinear, OutputLiner, QMask, KMask)
    res = run_bass_kernel_spmd(nc, in_maps, list(range(B)))
    out = np.stack([np.asarray(res.results[i]["Y"]) for i in range(B)])
    return out.astype(np.float32)


# revision 22
# speedup vs baseline: 1.3663x; 1.3663x over previous
"""Trainium2 Bass kernel for the nn_MultiHeadAttention problem.

Data-parallel over batch: each of the 8 NeuronCores processes one batch
element independently (no collectives).

Per-core dataflow (L=1024, E=1024, H=16, D=64; q/k tiles of 128,
e-chunks of 128 = 2 heads):

  host:  QT/KT/VT = Q[b].T etc in bf16 (no on-chip transposes),
         W2[c] = blockdiag(HeadLinear[2c], HeadLinear[2c+1]) bf16,
         masks as f32 [128, 8] in tile layout, O in bf16.
  proj:  qT2/kT2 [128,1024] bf16 = W2[c].T @ QT_chunk (both heads at once)
         v2 [128k,130] bf16 per k-tile = (VT_chunk.T @ W2[c]) * kmask,
         plus a kmask "ones" column per head.
  scores: s[k,q] psum = kT_h_slice.T @ qT_h (bf16); P = exp(s/8) via one
         ACT op per [128,1024] tile, output bf16.  No max subtraction
         (|s|/8 <~ 13), no explicit -1e10 mask: masked keys have v-rows
         and ones-column zeroed, which reproduces masked_fill+softmax
         exactly (exp(-1e10) == 0 in f32).
  PV:    out[65,q] psum = sum_k v2_slice.T @ P_slice (bf16, 512-wide
         moving operand); row 64 is the softmax denominator.  The psum
         tile is evacuated immediately (denom row -> dstack, rows 0:64
         -> ct unnormalized); reciprocals run in two [8,1024] batches
         off the critical path, then per-head partition_broadcast
         (GpSimd) + one DVE multiply normalizes ct in place.
  final: Y[q,:] psum = sum_c CT_slice.T @ O_chunk (bf16), multiplied by
         the query mask during psum->sbuf evacuation, DMA'd out.
"""

import os
import sys

import numpy as np

try:
    import concourse  # noqa: F401
except ImportError:  # pragma: no cover
    for _p in ("/opt/trn_rl_repo", os.path.expanduser("~/.axon_site/_ro/trn_rl_repo")):
        if os.path.isdir(_p) and _p not in sys.path:
            sys.path.insert(0, _p)

import ml_dtypes

import concourse.bass as bass
import concourse.tile as tile
from concourse import bacc, mybir

B, L, E, H, D = 8, 1024, 1024, 16, 64
P = 128          # partitions
NT = L // P      # 8 q/k tiles
NCH = E // P     # 8 e-chunks (2 heads each)
F32 = mybir.dt.float32
BF16 = mybir.dt.bfloat16


def build_bass():
    nc = bacc.Bacc(None, target_bir_lowering=False, debug=False)

    QT = nc.declare_dram_parameter("QT", [E, L], BF16, isOutput=False)
    KT = nc.declare_dram_parameter("KT", [E, L], BF16, isOutput=False)
    VT = nc.declare_dram_parameter("VT", [E, L], BF16, isOutput=False)
    W2 = nc.declare_dram_parameter("W2", [P, NCH, P], BF16, isOutput=False)
    OB = nc.declare_dram_parameter("OB", [E, E], BF16, isOutput=False)
    KM = nc.declare_dram_parameter("KM", [P, NT], F32, isOutput=False)
    QM = nc.declare_dram_parameter("QM", [P, NT], F32, isOutput=False)
    Y = nc.declare_dram_parameter("Y", [L, E], F32, isOutput=True)
    rbounce = nc.dram_tensor("rbounce", [H, L], BF16)

    with tile.TileContext(nc) as tc:
        with (
            tc.tile_pool(name="singles", bufs=1) as singles,
            tc.tile_pool(name="qkT", bufs=2) as qkT,
            tc.tile_pool(name="vaug", bufs=2) as vaug,
            tc.tile_pool(name="ppool", bufs=2) as ppool,
            tc.tile_pool(name="ystage", bufs=2) as ystage,
            tc.tile_pool(name="bcpool", bufs=3) as bcpool,
            tc.tile_pool(name="dtpool", bufs=2) as dtpool,
            tc.tile_pool(name="psbig", bufs=2, space="PSUM") as psbig,
            tc.tile_pool(name="pspv", bufs=1, space="PSUM") as pspv,
            tc.tile_pool(name="pssmall", bufs=2, space="PSUM") as pssmall,
        ):
            # --- persistent SBUF tensors -------------------------------
            qts = singles.tile([P, NCH, L], BF16)
            kts = singles.tile([P, NCH, L], BF16)
            vts = singles.tile([P, NCH, L], BF16)
            obs = singles.tile([P, NCH, E], BF16)
            w2s = singles.tile([P, NCH, P], BF16)
            kms = singles.tile([P, NT], F32)
            qms = singles.tile([P, NT], F32)
            ct = singles.tile([P, NCH, L], BF16)
            dstack0 = singles.tile([8, L], F32)
            dstack1 = singles.tile([8, L], F32)
            rstack0 = singles.tile([8, L], BF16)
            rstack1 = singles.tile([8, L], BF16)
            dstack = [dstack0, dstack1]
            rstack = [rstack0, rstack1]

            # --- input DMAs (small/consts first, then per-chunk) -------
            nc.gpsimd.dma_start(out=w2s[:], in_=W2[:])
            nc.gpsimd.dma_start(out=kms[:], in_=KM[:])
            nc.gpsimd.dma_start(out=qms[:], in_=QM[:])
            for c in range(NCH):
                nc.gpsimd.dma_start(out=qts[:, c, :], in_=QT[c * P:(c + 1) * P, :])
                nc.gpsimd.dma_start(out=kts[:, c, :], in_=KT[c * P:(c + 1) * P, :])
                nc.gpsimd.dma_start(out=vts[:, c, :], in_=VT[c * P:(c + 1) * P, :])
            for c in range(NCH):
                nc.gpsimd.dma_start(out=obs[:, c, :], in_=OB[c * P:(c + 1) * P, :])

            def normalize_heads(hs):
                batch = hs[0] // 8
                nc.gpsimd.dma_start(
                    out=rbounce[hs[0]:hs[-1] + 1, :], in_=rstack[batch][:])
                for h in hs:
                    c, hf = h // 2, h % 2
                    bcs = bcpool.tile([P, L], BF16)
                    src = rbounce[h:h + 1, :]
                    bc_in = bass.AP(
                        tensor=src.tensor, offset=src.offset,
                        ap=[[0, P], list(src.ap[-1])])
                    nc.gpsimd.dma_start(out=bcs[:], in_=bc_in)
                    sl = ct[64 * hf:64 * hf + 64, c, :]
                    nc.vector.tensor_mul(sl, sl, bcs[64 * hf:64 * hf + 64, :])

            # --- main loop over e-chunks (2 heads each) ----------------
            for c in range(NCH):
                # projections for both heads of this chunk
                qt2 = qkT.tile([P, L], BF16, tag="qt2")
                kt2 = qkT.tile([P, L], BF16, tag="kt2")
                for dst, src in ((qt2, qts), (kt2, kts)):
                    for hf in range(2):
                        ps = pssmall.tile([P, 512], F32, tag="small")
                        nc.tensor.matmul(
                            out=ps[:],
                            lhsT=w2s[:, c, :],
                            rhs=src[:, c, 512 * hf:512 * (hf + 1)],
                            start=True, stop=True,
                        )
                        nc.vector.tensor_copy(dst[:, 512 * hf:512 * (hf + 1)], ps[:])

                # v projection + key-mask + ones-column, both heads
                v2 = vaug.tile([P, NT, 130], BF16)
                for t in range(NT):
                    ps = pssmall.tile([P, P], F32, tag="small")
                    nc.tensor.matmul(
                        out=ps[:],
                        lhsT=vts[:, c, t * P:(t + 1) * P],
                        rhs=w2s[:, c, :],
                        start=True, stop=True,
                    )
                    nc.vector.tensor_scalar(
                        out=v2[:, t, 0:64], in0=ps[:, 0:64],
                        scalar1=kms[:, t:t + 1], scalar2=None,
                        op0=mybir.AluOpType.mult,
                    )
                    nc.vector.tensor_scalar(
                        out=v2[:, t, 65:129], in0=ps[:, 64:128],
                        scalar1=kms[:, t:t + 1], scalar2=None,
                        op0=mybir.AluOpType.mult,
                    )
                # denominator "ones" columns = key mask itself
                nc.vector.tensor_copy(v2[:, :, 64], kms[:, :])
                nc.vector.tensor_copy(v2[:, :, 129], kms[:, :])

                for hf in range(2):
                    h = 2 * c + hf
                    hq = qt2[64 * hf:64 * hf + 64, :]
                    hk = kt2[64 * hf:64 * hf + 64, :]
                    # scores (transposed, [k, q]) + exp -> P (bf16)
                    pt = ppool.tile([P, NT, L], BF16)
                    for t in range(NT):
                        sps = psbig.tile([P, L], F32, tag="big")
                        for qh in range(2):
                            nc.tensor.matmul(
                                out=sps[:, 512 * qh:512 * (qh + 1)],
                                lhsT=hk[:, t * P:(t + 1) * P],
                                rhs=hq[:, 512 * qh:512 * (qh + 1)],
                                start=True, stop=True,
                            )
                        nc.scalar.activation(
                            out=pt[:, t, :], in_=sps[:],
                            func=mybir.ActivationFunctionType.Exp,
                            scale=0.125,
                        )
                    # PV: out[65, q] accumulated over k-tiles, wide rhs
                    pv = pspv.tile([65, L], F32)
                    for kt in range(NT):
                        for qh in range(2):
                            nc.tensor.matmul(
                                out=pv[:, 512 * qh:512 * (qh + 1)],
                                lhsT=v2[:, kt, 65 * hf:65 * hf + 65],
                                rhs=pt[:, kt, 512 * qh:512 * (qh + 1)],
                                start=(kt == 0), stop=(kt == NT - 1),
                            )
                    # fast evacuation: denom row + unnormalized C^T rows
                    dtmp = dtpool.tile([65, L], F32)
                    nc.vector.tensor_copy(dtmp[64:65, :], pv[64:65, :])
                    nc.gpsimd.dma_start(
                        out=dstack[h // 8][h % 8:h % 8 + 1, :],
                        in_=dtmp[64:65, :])
                    nc.vector.tensor_copy(ct[64 * hf:64 * hf + 64, c, :], pv[0:64, :])

                if c in (3, 7):
                    batch = c // 4
                    with nc.allow_low_precision(reason="softmax recip bf16"):
                        nc.vector.reciprocal(
                            out=rstack[batch][:], in_=dstack[batch][:])
                    normalize_heads(range(8 * batch, 8 * batch + 8))

            # --- output projection ------------------------------------
            for t in range(NT):
                yps = psbig.tile([P, E], F32, tag="big")
                for c in range(NCH):
                    for eh in range(2):
                        nc.tensor.matmul(
                            out=yps[:, 512 * eh:512 * (eh + 1)],
                            lhsT=ct[:, c, t * P:(t + 1) * P],
                            rhs=obs[:, c, 512 * eh:512 * (eh + 1)],
                            start=(c == 0), stop=(c == NCH - 1),
                        )
                ys = ystage.tile([P, E], F32)
                nc.vector.tensor_scalar(
                    out=ys[:], in0=yps[:],
                    scalar1=qms[:, t:t + 1], scalar2=None,
                    op0=mybir.AluOpType.mult,
                )
                nc.gpsimd.dma_start(out=Y[t * P:(t + 1) * P, :], in_=ys[:])

    nc.compile()
    return nc


def make_core_inputs(Q, K, V, HeadLinear, OutputLiner, QMask, KMask):
    """Host-side sharding/layout prep. Returns list of per-core in_maps."""
    bf16 = ml_dtypes.bfloat16
    w2 = np.zeros((P, NCH, P), dtype=np.float32)
    hl = np.asarray(HeadLinear, dtype=np.float32)
    for c in range(NCH):
        w2[0:64, c, 0:64] = hl[2 * c]
        w2[64:128, c, 64:128] = hl[2 * c + 1]
    w2b = w2.astype(bf16)
    ob = np.asarray(OutputLiner, dtype=np.float32).astype(bf16)

    in_maps = []
    for b in range(B):
        qt = np.asarray(Q[b], dtype=np.float32).T.astype(bf16)
        kt = np.asarray(K[b], dtype=np.float32).T.astype(bf16)
        vt = np.asarray(V[b], dtype=np.float32).T.astype(bf16)
        km = np.ascontiguousarray(
            np.asarray(KMask[b]).astype(np.float32).reshape(NT, P).T)
        qm = np.ascontiguousarray(
            np.asarray(QMask[b]).astype(np.float32).reshape(NT, P).T)
        in_maps.append({
            "QT": np.ascontiguousarray(qt), "KT": np.ascontiguousarray(kt),
            "VT": np.ascontiguousarray(vt),
            "W2": w2b, "OB": ob,
            "KM": km, "QM": qm,
        })
    return in_maps


_NC_CACHE = None


def _get_nc():
    global _NC_CACHE
    if _NC_CACHE is None:
        _NC_CACHE = build_bass()
    return _NC_CACHE


def kernel(Q, K, V, HeadLinear, OutputLiner, QMask, KMask):
    from concourse.bass_utils import run_bass_kernel_spmd

    nc = _get_nc()
    in_maps = make_core_inputs(Q, K, V, HeadLinear, OutputLiner, QMask, KMask)
    res = run_bass_kernel_spmd(nc, in_maps, list(range(B)))
    out = np.stack([np.asarray(res.results[i]["Y"]) for i in range(B)])
    return out.astype(np.float32)


# revision 23
# speedup vs baseline: 1.4016x; 1.0258x over previous
"""Trainium2 Bass kernel for the nn_MultiHeadAttention problem.

Data-parallel over batch: each of the 8 NeuronCores processes one batch
element independently (no collectives).

Per-core dataflow (L=1024, E=1024, H=16, D=64; q/k tiles of 128,
e-chunks of 128 = 2 heads):

  host:  QT/KT/VT = Q[b].T etc in bf16 (no on-chip transposes); VT is
         pre-multiplied by the key mask, W2[c] = blockdiag(HL[2c],
         HL[2c+1]) bf16, masks as f32 [128, 8] tile layout, O in bf16.
  proj:  qT2/kT2 [128,1024] bf16 = W2[c].T @ QT_chunk (both heads)
         v2 [128k,130] bf16 per k-tile = VT_chunk.T @ W2[c] (already
         key-masked), plus a kmask "ones" column per head.
  scores: s[k,q] psum = kT_h_slice.T @ qT_h (bf16); P = exp(s/8) via one
         ACT op per [128,1024] tile, output bf16.  No max subtraction
         (|s|/8 <~ 13), no explicit -1e10 mask: masked keys have v-rows
         and ones-column zeroed, which reproduces masked_fill+softmax
         exactly (exp(-1e10) == 0 in f32).
  PV:    out[65,q] psum = sum_k v2_slice.T @ P_slice (bf16, 512-wide
         moving operand); row 64 is the softmax denominator.  The psum
         tile is evacuated immediately (denom row -> dstack via a
         partition-64 staging copy + DMA shuffle, rows 0:64 -> ct
         unnormalized); reciprocals run in three batches (heads 0-7,
         8-13, 14-15) off the critical path, then per-head DRAM-bounce
         broadcast + one DVE multiply normalizes ct in place.
  final: split output projection.  Part A (chunks 0-6) is emitted after
         the second normalize batch so it fills PE gaps during chunk 7;
         it accumulates into ysum (SBUF) with the query mask applied.
         Part B adds chunk 7 and writes Y.
"""

import os
import sys

import numpy as np

try:
    import concourse  # noqa: F401
except ImportError:  # pragma: no cover
    for _p in ("/opt/trn_rl_repo", os.path.expanduser("~/.axon_site/_ro/trn_rl_repo")):
        if os.path.isdir(_p) and _p not in sys.path:
            sys.path.insert(0, _p)

import ml_dtypes

import concourse.bass as bass
import concourse.tile as tile
from concourse import bacc, mybir

B, L, E, H, D = 8, 1024, 1024, 16, 64
P = 128          # partitions
NT = L // P      # 8 q/k tiles
NCH = E // P     # 8 e-chunks (2 heads each)
F32 = mybir.dt.float32
BF16 = mybir.dt.bfloat16

# normalize batches: (head range start, end, after-chunk)
NORM_BATCHES = [(0, 8, 3), (8, 14, 6), (14, 16, 7)]


def build_bass():
    nc = bacc.Bacc(None, target_bir_lowering=False, debug=False)

    QT = nc.declare_dram_parameter("QT", [E, L], BF16, isOutput=False)
    KT = nc.declare_dram_parameter("KT", [E, L], BF16, isOutput=False)
    VT = nc.declare_dram_parameter("VT", [E, L], BF16, isOutput=False)
    W2 = nc.declare_dram_parameter("W2", [P, NCH, P], BF16, isOutput=False)
    OB = nc.declare_dram_parameter("OB", [E, E], BF16, isOutput=False)
    KM = nc.declare_dram_parameter("KM", [P, NT], F32, isOutput=False)
    QM = nc.declare_dram_parameter("QM", [P, NT], F32, isOutput=False)
    Y = nc.declare_dram_parameter("Y", [L, E], F32, isOutput=True)
    rbounce = nc.dram_tensor("rbounce", [H, L], BF16)

    with tile.TileContext(nc) as tc:
        with (
            tc.tile_pool(name="singles", bufs=1) as singles,
            tc.tile_pool(name="qkT", bufs=2) as qkT,
            tc.tile_pool(name="vaug", bufs=2) as vaug,
            tc.tile_pool(name="ppool", bufs=2) as ppool,
            tc.tile_pool(name="ystage", bufs=2) as ystage,
            tc.tile_pool(name="bcpool", bufs=3) as bcpool,
            tc.tile_pool(name="dtpool", bufs=2) as dtpool,
            tc.tile_pool(name="psbig", bufs=2, space="PSUM") as psbig,
            tc.tile_pool(name="pspv", bufs=1, space="PSUM") as pspv,
            tc.tile_pool(name="pssmall", bufs=2, space="PSUM") as pssmall,
        ):
            # --- persistent SBUF tensors -------------------------------
            qts = singles.tile([P, NCH, L], BF16)
            kts = singles.tile([P, NCH, L], BF16)
            vts = singles.tile([P, NCH, L], BF16)
            obs = singles.tile([P, NCH, E], BF16)
            w2s = singles.tile([P, NCH, P], BF16)
            kms = singles.tile([P, NT], F32)
            qms = singles.tile([P, NT], F32)
            ct = singles.tile([P, NCH, L], BF16)
            ysum = singles.tile([P, NT, E], F32)
            dstacks = []
            rstacks = []
            for bi, (h0, h1, _) in enumerate(NORM_BATCHES):
                ds = singles.tile([h1 - h0, L], F32, tag=f"ds{bi}")
                rs = singles.tile([h1 - h0, L], BF16, tag=f"rs{bi}")
                dstacks.append(ds)
                rstacks.append(rs)

            # --- input DMAs (small/consts first, then per-chunk) -------
            nc.gpsimd.dma_start(out=w2s[:], in_=W2[:])
            nc.gpsimd.dma_start(out=kms[:], in_=KM[:])
            nc.gpsimd.dma_start(out=qms[:], in_=QM[:])
            for c in range(NCH):
                nc.gpsimd.dma_start(out=qts[:, c, :], in_=QT[c * P:(c + 1) * P, :])
                nc.gpsimd.dma_start(out=kts[:, c, :], in_=KT[c * P:(c + 1) * P, :])
                nc.gpsimd.dma_start(out=vts[:, c, :], in_=VT[c * P:(c + 1) * P, :])
            for c in range(NCH):
                nc.gpsimd.dma_start(out=obs[:, c, :], in_=OB[c * P:(c + 1) * P, :])

            def normalize_batch(bi):
                h0, h1, _ = NORM_BATCHES[bi]
                with nc.allow_low_precision(reason="softmax recip bf16"):
                    nc.vector.reciprocal(out=rstacks[bi][:], in_=dstacks[bi][:])
                nc.gpsimd.dma_start(out=rbounce[h0:h1, :], in_=rstacks[bi][:])
                for h in range(h0, h1):
                    c, hf = h // 2, h % 2
                    bcs = bcpool.tile([P, L], BF16)
                    src = rbounce[h:h + 1, :]
                    bc_in = bass.AP(
                        tensor=src.tensor, offset=src.offset,
                        ap=[[0, P], list(src.ap[-1])])
                    nc.gpsimd.dma_start(out=bcs[:], in_=bc_in)
                    sl = ct[64 * hf:64 * hf + 64, c, :]
                    nc.vector.tensor_mul(sl, sl, bcs[64 * hf:64 * hf + 64, :])

            def final_mms(t, yps, crange, start, stop):
                for c in crange:
                    for eh in range(2):
                        nc.tensor.matmul(
                            out=yps[:, 512 * eh:512 * (eh + 1)],
                            lhsT=ct[:, c, t * P:(t + 1) * P],
                            rhs=obs[:, c, 512 * eh:512 * (eh + 1)],
                            start=(c == crange[0]) and start,
                            stop=(c == crange[-1]) and stop,
                        )

            # --- main loop over e-chunks (2 heads each) ----------------
            for c in range(NCH):
                # projections for both heads of this chunk
                qt2 = qkT.tile([P, L], BF16, tag="qt2")
                kt2 = qkT.tile([P, L], BF16, tag="kt2")
                for dst, src in ((qt2, qts), (kt2, kts)):
                    for hf in range(2):
                        ps = pssmall.tile([P, 512], F32, tag="small")
                        nc.tensor.matmul(
                            out=ps[:],
                            lhsT=w2s[:, c, :],
                            rhs=src[:, c, 512 * hf:512 * (hf + 1)],
                            start=True, stop=True,
                        )
                        nc.vector.tensor_copy(dst[:, 512 * hf:512 * (hf + 1)], ps[:])

                # v projection (VT pre-masked on host), both heads
                v2 = vaug.tile([P, NT, 130], BF16)
                for t in range(NT):
                    ps = pssmall.tile([P, P], F32, tag="small")
                    nc.tensor.matmul(
                        out=ps[:],
                        lhsT=vts[:, c, t * P:(t + 1) * P],
                        rhs=w2s[:, c, :],
                        start=True, stop=True,
                    )
                    vt_out = bass.AP(
                        tensor=v2[:, t, 0:64].tensor,
                        offset=v2[:, t, 0:64].offset,
                        ap=[list(v2[:, t, 0:64].ap[0]), [65, 2], [1, 64]])
                    nc.vector.tensor_copy(
                        vt_out, ps[:].rearrange("p (two d) -> p two d", two=2))
                # denominator "ones" columns = key mask itself
                nc.vector.tensor_copy(v2[:, :, 64], kms[:, :])
                nc.vector.tensor_copy(v2[:, :, 129], kms[:, :])

                for hf in range(2):
                    h = 2 * c + hf
                    hq = qt2[64 * hf:64 * hf + 64, :]
                    hk = kt2[64 * hf:64 * hf + 64, :]
                    # scores (transposed, [k, q]) + exp -> P (bf16)
                    pt = ppool.tile([P, NT, L], BF16)
                    for t in range(NT):
                        sps = psbig.tile([P, L], F32, tag="big")
                        for qh in range(2):
                            nc.tensor.matmul(
                                out=sps[:, 512 * qh:512 * (qh + 1)],
                                lhsT=hk[:, t * P:(t + 1) * P],
                                rhs=hq[:, 512 * qh:512 * (qh + 1)],
                                start=True, stop=True,
                            )
                        nc.scalar.activation(
                            out=pt[:, t, :], in_=sps[:],
                            func=mybir.ActivationFunctionType.Exp,
                            scale=0.125,
                        )
                    # PV: out[65, q] accumulated over k-tiles, wide rhs
                    pv = pspv.tile([65, L], F32)
                    for kt in range(NT):
                        for qh in range(2):
                            nc.tensor.matmul(
                                out=pv[:, 512 * qh:512 * (qh + 1)],
                                lhsT=v2[:, kt, 65 * hf:65 * hf + 65],
                                rhs=pt[:, kt, 512 * qh:512 * (qh + 1)],
                                start=(kt == 0), stop=(kt == NT - 1),
                            )
                    # fast evacuation: denom row + unnormalized C^T rows
                    dtmp = dtpool.tile([65, L], F32)
                    nc.vector.tensor_copy(dtmp[64:65, :], pv[64:65, :])
                    bi = next(i for i, (a, b, _) in enumerate(NORM_BATCHES)
                              if a <= h < b)
                    nc.gpsimd.dma_start(
                        out=dstacks[bi][h - NORM_BATCHES[bi][0]:
                                        h - NORM_BATCHES[bi][0] + 1, :],
                        in_=dtmp[64:65, :])
                    nc.vector.tensor_copy(ct[64 * hf:64 * hf + 64, c, :], pv[0:64, :])

                for bi, (_, _, bc_) in enumerate(NORM_BATCHES):
                    if c == bc_ and bi < 2:
                        normalize_batch(bi)

            # tail: last normalize batch, then the split output projection
            normalize_batch(2)

            # part A: chunks 0-6 (all normalized after batch 1), absorbs
            # PE idle while chunk 7 compute and batch-2 normalize run
            for t in range(NT):
                yps = psbig.tile([P, E], F32, tag="big")
                final_mms(t, yps, list(range(7)), start=True, stop=True)
                nc.vector.tensor_scalar(
                    out=ysum[:, t, :], in0=yps[:],
                    scalar1=qms[:, t:t + 1], scalar2=None,
                    op0=mybir.AluOpType.mult,
                )
            # part B: chunk 7 + combine
            for t in range(NT):
                yps = psbig.tile([P, E], F32, tag="big")
                final_mms(t, yps, [7], start=True, stop=True)
                yt = ystage.tile([P, E], F32, tag="yt")
                nc.vector.tensor_scalar(
                    out=yt[:], in0=yps[:],
                    scalar1=qms[:, t:t + 1], scalar2=None,
                    op0=mybir.AluOpType.mult,
                )
                ys = ystage.tile([P, E], F32, tag="ys")
                nc.vector.tensor_add(ys[:], yt[:], ysum[:, t, :])
                nc.gpsimd.dma_start(out=Y[t * P:(t + 1) * P, :], in_=ys[:])

    nc.compile()
    return nc


def make_core_inputs(Q, K, V, HeadLinear, OutputLiner, QMask, KMask):
    """Host-side sharding/layout prep. Returns list of per-core in_maps."""
    bf16 = ml_dtypes.bfloat16
    w2 = np.zeros((P, NCH, P), dtype=np.float32)
    hl = np.asarray(HeadLinear, dtype=np.float32)
    for c in range(NCH):
        w2[0:64, c, 0:64] = hl[2 * c]
        w2[64:128, c, 64:128] = hl[2 * c + 1]
    w2b = w2.astype(bf16)
    ob = np.asarray(OutputLiner, dtype=np.float32).astype(bf16)

    in_maps = []
    for b in range(B):
        kmask = np.asarray(KMask[b]).astype(np.float32)
        qt = np.asarray(Q[b], dtype=np.float32).T.astype(bf16)
        kt = np.asarray(K[b], dtype=np.float32).T.astype(bf16)
        vmasked = np.asarray(V[b], dtype=np.float32) * kmask[:, None]
        vt = vmasked.T.astype(bf16)
        km = np.ascontiguousarray(kmask.reshape(NT, P).T)
        qm = np.ascontiguousarray(
            np.asarray(QMask[b]).astype(np.float32).reshape(NT, P).T)
        in_maps.append({
            "QT": np.ascontiguousarray(qt), "KT": np.ascontiguousarray(kt),
            "VT": np.ascontiguousarray(vt),
            "W2": w2b, "OB": ob,
            "KM": km, "QM": qm,
        })
    return in_maps


_NC_CACHE = None


def _get_nc():
    global _NC_CACHE
    if _NC_CACHE is None:
        _NC_CACHE = build_bass()
    return _NC_CACHE


def kernel(Q, K, V, HeadLinear, OutputLiner, QMask, KMask):
    from concourse.bass_utils import run_bass_kernel_spmd

    nc = _get_nc()
    in_maps = make_core_inputs(Q, K, V, HeadLinear, OutputLiner, QMask, KMask)
    res = run_bass_kernel_spmd(nc, in_maps, list(range(B)))
    out = np.stack([np.asarray(res.results[i]["Y"]) for i in range(B)])
    return out.astype(np.float32)


# revision 24
# speedup vs baseline: 1.9605x; 1.3988x over previous
"""Trainium2 Bass kernel for the nn_MultiHeadAttention problem.

Data-parallel over batch: each of the 8 NeuronCores processes one batch
element independently (no collectives).

Mask compaction: the host gathers only the valid query/key positions
(QMask/KMask true), padded to a multiple of 128, and scatters the
output back (masked query rows are exactly zero in the reference).
With ~50% random masks this cuts the attention work ~4x.  The tile
counts (ntq, ntk) are chosen from the actual masks at kernel() time and
a bass program is compiled per shape, so any mask density works.

Per-core dataflow (E=1024, H=16, D=64; Lq=ntq*128 queries, Lk=ntk*128
keys after compaction; e-chunks of 128 = 2 heads):

  host:  QTc/KTc/VTc = compacted Q[b].T etc in bf16 (pad columns zero),
         W2[c] = blockdiag(HL[2c], HL[2c+1]) bf16, KM = compacted-slot
         validity f32, O in bf16.
  proj:  qT2 [128,Lq] / kT2 [128,Lk] bf16 = W2[c].T @ QTc_chunk
         v2 [128k,130] bf16 per k-tile = VTc_chunk.T @ W2[c], plus a
         validity "ones" column per head.
  scores: s[k,q] psum = kT_h_slice.T @ qT_h (bf16); P = exp(s/8) via one
         ACT op per [128,Lq] tile, output bf16.  No max subtraction
         (|s|/8 <~ 13); masked/pad keys have v-rows and ones-column
         zeroed, reproducing masked_fill+softmax exactly.
  PV:    out[65,q] psum = sum_k v2_slice.T @ P_slice (bf16); row 64 is
         the softmax denominator.  Fast psum evacuation (denom row ->
         dstack via partition-64 staging + DMA shuffle, rows 0:64 -> ct
         unnormalized); reciprocals in three batches off the critical
         path, then DRAM-bounce broadcast + one DVE multiply per head
         normalizes ct in place.
  final: split output projection: part A (chunks 0-6) fills PE gaps
         during chunk 7, accumulating to ysum (SBUF); part B adds
         chunk 7 and writes Y.
"""

import math
import os
import sys

import numpy as np

try:
    import concourse  # noqa: F401
except ImportError:  # pragma: no cover
    for _p in ("/opt/trn_rl_repo", os.path.expanduser("~/.axon_site/_ro/trn_rl_repo")):
        if os.path.isdir(_p) and _p not in sys.path:
            sys.path.insert(0, _p)

import ml_dtypes

import concourse.bass as bass
import concourse.tile as tile
from concourse import bacc, mybir

B, L, E, H, D = 8, 1024, 1024, 16, 64
P = 128          # partitions
NCH = E // P     # 8 e-chunks (2 heads each)
F32 = mybir.dt.float32
BF16 = mybir.dt.bfloat16

# normalize batches: (head range start, end, after-chunk)
NORM_BATCHES = [(0, 8, 3), (8, 14, 6), (14, 16, 7)]


def _chunks(n, step=512):
    """Split [0, n) into [start, end) pieces of at most `step`."""
    return [(s, min(s + step, n)) for s in range(0, n, step)]


def build_bass(ntq, ntk):
    Lq, Lk = ntq * P, ntk * P
    nc = bacc.Bacc(None, target_bir_lowering=False, debug=False)

    QT = nc.declare_dram_parameter("QT", [E, Lq], BF16, isOutput=False)
    KT = nc.declare_dram_parameter("KT", [E, Lk], BF16, isOutput=False)
    VT = nc.declare_dram_parameter("VT", [E, Lk], BF16, isOutput=False)
    W2 = nc.declare_dram_parameter("W2", [P, NCH, P], BF16, isOutput=False)
    OB = nc.declare_dram_parameter("OB", [E, E], BF16, isOutput=False)
    KM = nc.declare_dram_parameter("KM", [P, ntk], F32, isOutput=False)
    Y = nc.declare_dram_parameter("Y", [Lq, E], F32, isOutput=True)
    rbounce = nc.dram_tensor("rbounce", [H, Lq], BF16)

    with tile.TileContext(nc) as tc:
        with (
            tc.tile_pool(name="singles", bufs=1) as singles,
            tc.tile_pool(name="qkT", bufs=2) as qkT,
            tc.tile_pool(name="vaug", bufs=2) as vaug,
            tc.tile_pool(name="ppool", bufs=2) as ppool,
            tc.tile_pool(name="ystage", bufs=2) as ystage,
            tc.tile_pool(name="bcpool", bufs=3) as bcpool,
            tc.tile_pool(name="dtpool", bufs=2) as dtpool,
            tc.tile_pool(name="psbig", bufs=2, space="PSUM") as psbig,
            tc.tile_pool(name="pspv", bufs=1, space="PSUM") as pspv,
            tc.tile_pool(name="pssmall", bufs=2, space="PSUM") as pssmall,
        ):
            # --- persistent SBUF tensors -------------------------------
            qts = singles.tile([P, NCH, Lq], BF16)
            kts = singles.tile([P, NCH, Lk], BF16)
            vts = singles.tile([P, NCH, Lk], BF16)
            obs = singles.tile([P, NCH, E], BF16)
            w2s = singles.tile([P, NCH, P], BF16)
            kms = singles.tile([P, ntk], F32)
            ct = singles.tile([P, NCH, Lq], BF16)
            ysum = singles.tile([P, ntq, E], F32)
            dstacks = []
            rstacks = []
            for bi, (h0, h1, _) in enumerate(NORM_BATCHES):
                ds = singles.tile([h1 - h0, Lq], F32, tag=f"ds{bi}")
                rs = singles.tile([h1 - h0, Lq], BF16, tag=f"rs{bi}")
                dstacks.append(ds)
                rstacks.append(rs)

            # --- input DMAs (small/consts first, then per-chunk) -------
            nc.gpsimd.dma_start(out=w2s[:], in_=W2[:])
            nc.gpsimd.dma_start(out=kms[:], in_=KM[:])
            for c in range(NCH):
                nc.gpsimd.dma_start(out=qts[:, c, :], in_=QT[c * P:(c + 1) * P, :])
                nc.gpsimd.dma_start(out=kts[:, c, :], in_=KT[c * P:(c + 1) * P, :])
                nc.gpsimd.dma_start(out=vts[:, c, :], in_=VT[c * P:(c + 1) * P, :])
            for c in range(NCH):
                nc.gpsimd.dma_start(out=obs[:, c, :], in_=OB[c * P:(c + 1) * P, :])

            def normalize_batch(bi):
                h0, h1, _ = NORM_BATCHES[bi]
                with nc.allow_low_precision(reason="softmax recip bf16"):
                    nc.vector.reciprocal(out=rstacks[bi][:], in_=dstacks[bi][:])
                nc.gpsimd.dma_start(out=rbounce[h0:h1, :], in_=rstacks[bi][:])
                for h in range(h0, h1):
                    c, hf = h // 2, h % 2
                    bcs = bcpool.tile([P, Lq], BF16)
                    src = rbounce[h:h + 1, :]
                    bc_in = bass.AP(
                        tensor=src.tensor, offset=src.offset,
                        ap=[[0, P], list(src.ap[-1])])
                    nc.gpsimd.dma_start(out=bcs[:], in_=bc_in)
                    sl = ct[64 * hf:64 * hf + 64, c, :]
                    nc.vector.tensor_mul(sl, sl, bcs[64 * hf:64 * hf + 64, :])

            def final_mms(t, yps, crange):
                for c in crange:
                    for eh in range(2):
                        nc.tensor.matmul(
                            out=yps[:, 512 * eh:512 * (eh + 1)],
                            lhsT=ct[:, c, t * P:(t + 1) * P],
                            rhs=obs[:, c, 512 * eh:512 * (eh + 1)],
                            start=(c == crange[0]), stop=(c == crange[-1]),
                        )

            # --- main loop over e-chunks (2 heads each) ----------------
            for c in range(NCH):
                # projections for both heads of this chunk
                qt2 = qkT.tile([P, Lq], BF16, tag="qt2")
                kt2 = qkT.tile([P, Lk], BF16, tag="kt2")
                for dst, src, ln in ((qt2, qts, Lq), (kt2, kts, Lk)):
                    for s0, s1 in _chunks(ln):
                        ps = pssmall.tile([P, 512], F32, tag="small")
                        nc.tensor.matmul(
                            out=ps[:, 0:s1 - s0],
                            lhsT=w2s[:, c, :],
                            rhs=src[:, c, s0:s1],
                            start=True, stop=True,
                        )
                        nc.vector.tensor_copy(dst[:, s0:s1], ps[:, 0:s1 - s0])

                # v projection (keys compacted: only validity col needed)
                v2 = vaug.tile([P, ntk, 130], BF16)
                for t in range(ntk):
                    ps = pssmall.tile([P, P], F32, tag="small")
                    nc.tensor.matmul(
                        out=ps[:],
                        lhsT=vts[:, c, t * P:(t + 1) * P],
                        rhs=w2s[:, c, :],
                        start=True, stop=True,
                    )
                    base = v2[:, t, 0:64]
                    vt_out = bass.AP(
                        tensor=base.tensor, offset=base.offset,
                        ap=[list(base.ap[0]), [65, 2], [1, 64]])
                    nc.vector.tensor_copy(
                        vt_out, ps[:].rearrange("p (two d) -> p two d", two=2))
                # denominator "ones" columns = slot-validity mask
                nc.vector.tensor_copy(v2[:, :, 64], kms[:, :])
                nc.vector.tensor_copy(v2[:, :, 129], kms[:, :])

                for hf in range(2):
                    h = 2 * c + hf
                    hq = qt2[64 * hf:64 * hf + 64, :]
                    hk = kt2[64 * hf:64 * hf + 64, :]
                    # scores (transposed, [k, q]) + exp -> P (bf16)
                    pt = ppool.tile([P, ntk, Lq], BF16)
                    for t in range(ntk):
                        sps = psbig.tile([P, Lq], F32, tag="big")
                        for s0, s1 in _chunks(Lq):
                            nc.tensor.matmul(
                                out=sps[:, s0:s1],
                                lhsT=hk[:, t * P:(t + 1) * P],
                                rhs=hq[:, s0:s1],
                                start=True, stop=True,
                            )
                        nc.scalar.activation(
                            out=pt[:, t, :], in_=sps[:],
                            func=mybir.ActivationFunctionType.Exp,
                            scale=0.125,
                        )
                    # PV: out[65, q] accumulated over k-tiles, wide rhs
                    pv = pspv.tile([65, Lq], F32)
                    for kt in range(ntk):
                        for s0, s1 in _chunks(Lq):
                            nc.tensor.matmul(
                                out=pv[:, s0:s1],
                                lhsT=v2[:, kt, 65 * hf:65 * hf + 65],
                                rhs=pt[:, kt, s0:s1],
                                start=(kt == 0), stop=(kt == ntk - 1),
                            )
                    # fast evacuation: denom row + unnormalized C^T rows
                    dtmp = dtpool.tile([65, Lq], F32)
                    nc.vector.tensor_copy(dtmp[64:65, :], pv[64:65, :])
                    bi = next(i for i, (a, b, _) in enumerate(NORM_BATCHES)
                              if a <= h < b)
                    nc.gpsimd.dma_start(
                        out=dstacks[bi][h - NORM_BATCHES[bi][0]:
                                        h - NORM_BATCHES[bi][0] + 1, :],
                        in_=dtmp[64:65, :])
                    nc.vector.tensor_copy(ct[64 * hf:64 * hf + 64, c, :], pv[0:64, :])

                for bi, (_, _, bc_) in enumerate(NORM_BATCHES):
                    if c == bc_ and bi < 2:
                        normalize_batch(bi)

            # tail: last normalize batch, then the split output projection
            normalize_batch(2)

            # part A: chunks 0-6 (all normalized after batch 1), absorbs
            # PE idle while chunk 7 compute and batch-2 normalize run
            for t in range(ntq):
                yps = psbig.tile([P, E], F32, tag="big")
                final_mms(t, yps, list(range(7)))
                nc.vector.tensor_copy(ysum[:, t, :], yps[:])
            # part B: chunk 7 + combine
            for t in range(ntq):
                yps = psbig.tile([P, E], F32, tag="big")
                final_mms(t, yps, [7])
                ys = ystage.tile([P, E], F32, tag="ys")
                nc.vector.tensor_add(ys[:], yps[:], ysum[:, t, :])
                nc.gpsimd.dma_start(out=Y[t * P:(t + 1) * P, :], in_=ys[:])

    nc.compile()
    return nc


def make_core_inputs(Q, K, V, HeadLinear, OutputLiner, QMask, KMask):
    """Host-side sharding/compaction. Returns (in_maps, qidxs, ntq, ntk)."""
    bf16 = ml_dtypes.bfloat16
    qm = np.asarray(QMask).astype(bool)
    km = np.asarray(KMask).astype(bool)
    qidxs = [np.nonzero(qm[b])[0] for b in range(B)]
    kidxs = [np.nonzero(km[b])[0] for b in range(B)]
    ntq = max(1, math.ceil(max(len(ix) for ix in qidxs) / P))
    ntk = max(1, math.ceil(max(len(ix) for ix in kidxs) / P))
    Lq, Lk = ntq * P, ntk * P

    w2 = np.zeros((P, NCH, P), dtype=np.float32)
    hl = np.asarray(HeadLinear, dtype=np.float32)
    for c in range(NCH):
        w2[0:64, c, 0:64] = hl[2 * c]
        w2[64:128, c, 64:128] = hl[2 * c + 1]
    w2b = w2.astype(bf16)
    ob = np.asarray(OutputLiner, dtype=np.float32).astype(bf16)

    in_maps = []
    for b in range(B):
        qi, ki = qidxs[b], kidxs[b]
        qc = np.zeros((Lq, E), dtype=np.float32)
        qc[:len(qi)] = np.asarray(Q[b], dtype=np.float32)[qi]
        kc = np.zeros((Lk, E), dtype=np.float32)
        kc[:len(ki)] = np.asarray(K[b], dtype=np.float32)[ki]
        vc = np.zeros((Lk, E), dtype=np.float32)
        vc[:len(ki)] = np.asarray(V[b], dtype=np.float32)[ki]
        kmc = np.zeros(Lk, dtype=np.float32)
        kmc[:len(ki)] = 1.0
        in_maps.append({
            "QT": np.ascontiguousarray(qc.T.astype(bf16)),
            "KT": np.ascontiguousarray(kc.T.astype(bf16)),
            "VT": np.ascontiguousarray(vc.T.astype(bf16)),
            "W2": w2b, "OB": ob,
            "KM": np.ascontiguousarray(kmc.reshape(ntk, P).T),
        })
    return in_maps, qidxs, ntq, ntk


_NC_CACHE = {}


def _get_nc(ntq, ntk):
    if (ntq, ntk) not in _NC_CACHE:
        _NC_CACHE[(ntq, ntk)] = build_bass(ntq, ntk)
    return _NC_CACHE[(ntq, ntk)]


def kernel(Q, K, V, HeadLinear, OutputLiner, QMask, KMask):
    from concourse.bass_utils import run_bass_kernel_spmd

    in_maps, qidxs, ntq, ntk = make_core_inputs(
        Q, K, V, HeadLinear, OutputLiner, QMask, KMask)
    nc = _get_nc(ntq, ntk)
    res = run_bass_kernel_spmd(nc, in_maps, list(range(B)))
    out = np.zeros((B, L, E), dtype=np.float32)
    for b in range(B):
        yc = np.asarray(res.results[b]["Y"])
        out[b][qidxs[b]] = yc[:len(qidxs[b])]
    return out


# revision 25
# speedup vs baseline: 2.1938x; 1.1190x over previous
"""Trainium2 Bass kernel for the nn_MultiHeadAttention problem.

Data-parallel over batch: each of the 8 NeuronCores processes one batch
element independently (no collectives).

Mask compaction: the host gathers only the valid query/key positions
(QMask/KMask true), padded to a multiple of 128, and scatters the
output back (masked query rows are exactly zero in the reference).
With ~50% random masks this cuts the attention work ~4x.  The tile
counts (ntq, ntk) are chosen from the actual masks at kernel() time and
a bass program is compiled per shape, so any mask density works.

Per-core dataflow (E=1024, H=16, D=64; Lq=ntq*128 queries, Lk=ntk*128
keys after compaction; e-chunks of 128 = 2 heads):

  host:  QTc/KTc/VTc = compacted Q[b].T etc in bf16 (pad columns zero),
         W2[c] = blockdiag(HL[2c], HL[2c+1]) bf16, KM = compacted-slot
         validity f32, O in bf16.
  proj:  qT2 [128,Lq] / kT2 [128,Lk] bf16 = W2[c].T @ QTc_chunk
         v2 [128k,130] bf16 per k-tile = VTc_chunk.T @ W2[c], plus a
         validity "ones" column per head.
  scores: s[k,q] psum = kT_h_slice.T @ qT_h (bf16); P = exp(s/8) via one
         ACT op per [128,Lq] tile, output bf16.  No max subtraction
         (|s|/8 <~ 13); masked/pad keys have v-rows and ones-column
         zeroed, reproducing masked_fill+softmax exactly.
  PV:    out[65,q] psum = sum_k v2_slice.T @ P_slice (bf16); row 64 is
         the softmax denominator.  Fast psum evacuation (denom row ->
         dstack via partition-64 staging + DMA shuffle, rows 0:64 -> ct
         unnormalized); reciprocals in three batches off the critical
         path, then DRAM-bounce broadcast + one DVE multiply per head
         normalizes ct in place.
  final: split output projection: part A (chunks 0-6) fills PE gaps
         during chunk 7, accumulating to ysum (SBUF); part B adds
         chunk 7 and writes Y.
"""

import math
import os
import sys

import numpy as np

try:
    import concourse  # noqa: F401
except ImportError:  # pragma: no cover
    for _p in ("/opt/trn_rl_repo", os.path.expanduser("~/.axon_site/_ro/trn_rl_repo")):
        if os.path.isdir(_p) and _p not in sys.path:
            sys.path.insert(0, _p)

import ml_dtypes

import concourse.bass as bass
import concourse.tile as tile
from concourse import bacc, mybir

B, L, E, H, D = 8, 1024, 1024, 16, 64
P = 128          # partitions
NCH = E // P     # 8 e-chunks (2 heads each)
F32 = mybir.dt.float32
BF16 = mybir.dt.bfloat16

# normalize batches: (head range start, end, after-chunk)
NORM_BATCHES = [(0, 8, 3), (8, 14, 6), (14, 16, 7)]


def _chunks(n, step=512):
    """Split [0, n) into [start, end) pieces of at most `step`."""
    return [(s, min(s + step, n)) for s in range(0, n, step)]


def build_bass(ntq, ntk):
    Lq, Lk = ntq * P, ntk * P
    nc = bacc.Bacc(None, target_bir_lowering=False, debug=False)

    QT = nc.declare_dram_parameter("QT", [E, Lq], BF16, isOutput=False)
    KT = nc.declare_dram_parameter("KT", [E, Lk], BF16, isOutput=False)
    VT = nc.declare_dram_parameter("VT", [E, Lk], BF16, isOutput=False)
    W2 = nc.declare_dram_parameter("W2", [P, NCH, P], BF16, isOutput=False)
    OB = nc.declare_dram_parameter("OB", [E, E], BF16, isOutput=False)
    KM = nc.declare_dram_parameter("KM", [P, ntk], F32, isOutput=False)
    Y = nc.declare_dram_parameter("Y", [Lq, E], F32, isOutput=True)
    rbounce = nc.dram_tensor("rbounce", [H, Lq], BF16)

    with tile.TileContext(nc) as tc:
        with (
            tc.tile_pool(name="singles", bufs=1) as singles,
            tc.tile_pool(name="qkT", bufs=2) as qkT,
            tc.tile_pool(name="vaug", bufs=2) as vaug,
            tc.tile_pool(name="ppool", bufs=2) as ppool,
            tc.tile_pool(name="ystage", bufs=2) as ystage,
            tc.tile_pool(name="bcpool", bufs=3) as bcpool,
            tc.tile_pool(name="dtpool", bufs=2) as dtpool,
            tc.tile_pool(name="psbig", bufs=2, space="PSUM") as psbig,
            tc.tile_pool(name="pspv", bufs=1, space="PSUM") as pspv,
            tc.tile_pool(name="pssmall", bufs=2, space="PSUM") as pssmall,
        ):
            # --- persistent SBUF tensors -------------------------------
            qts = singles.tile([P, NCH, Lq], BF16)
            kts = singles.tile([P, NCH, Lk], BF16)
            vts = singles.tile([P, NCH, Lk], BF16)
            obs = singles.tile([P, NCH, E], BF16)
            w2s = singles.tile([P, NCH, P], BF16)
            kms = singles.tile([P, ntk], F32)
            ct = singles.tile([P, NCH, Lq], BF16)
            ysum = singles.tile([P, ntq, E], F32)
            dstacks = []
            rstacks = []
            for bi, (h0, h1, _) in enumerate(NORM_BATCHES):
                ds = singles.tile([h1 - h0, Lq], F32, tag=f"ds{bi}")
                rs = singles.tile([h1 - h0, Lq], BF16, tag=f"rs{bi}")
                dstacks.append(ds)
                rstacks.append(rs)

            # --- input DMAs (small/consts first, then per-chunk) -------
            nc.gpsimd.dma_start(out=w2s[:], in_=W2[:])
            nc.gpsimd.dma_start(out=kms[:], in_=KM[:])
            for c in range(NCH):
                nc.sync.dma_start(out=qts[:, c, :], in_=QT[c * P:(c + 1) * P, :])
                nc.sync.dma_start(out=kts[:, c, :], in_=KT[c * P:(c + 1) * P, :])
                nc.sync.dma_start(out=vts[:, c, :], in_=VT[c * P:(c + 1) * P, :])
            for c in range(NCH):
                nc.sync.dma_start(out=obs[:, c, :], in_=OB[c * P:(c + 1) * P, :])

            def normalize_batch(bi):
                h0, h1, _ = NORM_BATCHES[bi]
                with nc.allow_low_precision(reason="softmax recip bf16"):
                    nc.vector.reciprocal(out=rstacks[bi][:], in_=dstacks[bi][:])
                nc.gpsimd.dma_start(out=rbounce[h0:h1, :], in_=rstacks[bi][:])
                for h in range(h0, h1):
                    c, hf = h // 2, h % 2
                    bcs = bcpool.tile([P, Lq], BF16)
                    src = rbounce[h:h + 1, :]
                    bc_in = bass.AP(
                        tensor=src.tensor, offset=src.offset,
                        ap=[[0, P], list(src.ap[-1])])
                    nc.gpsimd.dma_start(out=bcs[:], in_=bc_in)
                    sl = ct[64 * hf:64 * hf + 64, c, :]
                    nc.vector.tensor_mul(sl, sl, bcs[64 * hf:64 * hf + 64, :])

            def final_mms(t, yps, crange):
                for c in crange:
                    for eh in range(2):
                        nc.tensor.matmul(
                            out=yps[:, 512 * eh:512 * (eh + 1)],
                            lhsT=ct[:, c, t * P:(t + 1) * P],
                            rhs=obs[:, c, 512 * eh:512 * (eh + 1)],
                            start=(c == crange[0]), stop=(c == crange[-1]),
                        )

            # --- main loop over e-chunks (2 heads each) ----------------
            for c in range(NCH):
                # projections for both heads of this chunk
                qt2 = qkT.tile([P, Lq], BF16, tag="qt2")
                kt2 = qkT.tile([P, Lk], BF16, tag="kt2")
                for dst, src, ln in ((qt2, qts, Lq), (kt2, kts, Lk)):
                    for s0, s1 in _chunks(ln):
                        ps = pssmall.tile([P, 512], F32, tag="small")
                        nc.tensor.matmul(
                            out=ps[:, 0:s1 - s0],
                            lhsT=w2s[:, c, :],
                            rhs=src[:, c, s0:s1],
                            start=True, stop=True,
                        )
                        nc.scalar.copy(dst[:, s0:s1], ps[:, 0:s1 - s0])

                # v projection (keys compacted: only validity col needed)
                v2 = vaug.tile([P, ntk, 130], BF16)
                for t in range(ntk):
                    ps = pssmall.tile([P, P], F32, tag="small")
                    nc.tensor.matmul(
                        out=ps[:],
                        lhsT=vts[:, c, t * P:(t + 1) * P],
                        rhs=w2s[:, c, :],
                        start=True, stop=True,
                    )
                    base = v2[:, t, 0:64]
                    vt_out = bass.AP(
                        tensor=base.tensor, offset=base.offset,
                        ap=[list(base.ap[0]), [65, 2], [1, 64]])
                    nc.scalar.copy(
                        vt_out, ps[:].rearrange("p (two d) -> p two d", two=2))
                # denominator "ones" columns = slot-validity mask
                nc.vector.tensor_copy(v2[:, :, 64], kms[:, :])
                nc.vector.tensor_copy(v2[:, :, 129], kms[:, :])

                for hf in range(2):
                    h = 2 * c + hf
                    hq = qt2[64 * hf:64 * hf + 64, :]
                    hk = kt2[64 * hf:64 * hf + 64, :]
                    # scores (transposed, [k, q]) + exp -> P (bf16)
                    pt = ppool.tile([P, ntk, Lq], BF16)
                    for t in range(ntk):
                        sps = psbig.tile([P, Lq], F32, tag="big")
                        for s0, s1 in _chunks(Lq):
                            nc.tensor.matmul(
                                out=sps[:, s0:s1],
                                lhsT=hk[:, t * P:(t + 1) * P],
                                rhs=hq[:, s0:s1],
                                start=True, stop=True,
                            )
                        nc.scalar.activation(
                            out=pt[:, t, :], in_=sps[:],
                            func=mybir.ActivationFunctionType.Exp,
                            scale=0.125,
                        )
                    # PV: out[65, q] accumulated over k-tiles, wide rhs
                    pv = pspv.tile([65, Lq], F32)
                    for kt in range(ntk):
                        for s0, s1 in _chunks(Lq):
                            nc.tensor.matmul(
                                out=pv[:, s0:s1],
                                lhsT=v2[:, kt, 65 * hf:65 * hf + 65],
                                rhs=pt[:, kt, s0:s1],
                                start=(kt == 0), stop=(kt == ntk - 1),
                            )
                    # fast evacuation: denom row + unnormalized C^T rows
                    dtmp = dtpool.tile([65, Lq], F32)
                    nc.vector.tensor_copy(dtmp[64:65, :], pv[64:65, :])
                    bi = next(i for i, (a, b, _) in enumerate(NORM_BATCHES)
                              if a <= h < b)
                    nc.gpsimd.dma_start(
                        out=dstacks[bi][h - NORM_BATCHES[bi][0]:
                                        h - NORM_BATCHES[bi][0] + 1, :],
                        in_=dtmp[64:65, :])
                    nc.vector.tensor_copy(ct[64 * hf:64 * hf + 64, c, :], pv[0:64, :])

                for bi, (_, _, bc_) in enumerate(NORM_BATCHES):
                    if c == bc_ and bi < 2:
                        normalize_batch(bi)

            # tail: last normalize batch, then the split output projection
            normalize_batch(2)

            # part A: chunks 0-6 (all normalized after batch 1), absorbs
            # PE idle while chunk 7 compute and batch-2 normalize run
            for t in range(ntq):
                yps = psbig.tile([P, E], F32, tag="big")
                final_mms(t, yps, list(range(7)))
                nc.vector.tensor_copy(ysum[:, t, :], yps[:])
            # part B: chunk 7 + combine
            for t in range(ntq):
                yps = psbig.tile([P, E], F32, tag="big")
                final_mms(t, yps, [7])
                ys = ystage.tile([P, E], F32, tag="ys")
                nc.vector.tensor_add(ys[:], yps[:], ysum[:, t, :])
                nc.gpsimd.dma_start(out=Y[t * P:(t + 1) * P, :], in_=ys[:])

    nc.compile()
    return nc


def make_core_inputs(Q, K, V, HeadLinear, OutputLiner, QMask, KMask):
    """Host-side sharding/compaction. Returns (in_maps, qidxs, ntq, ntk)."""
    bf16 = ml_dtypes.bfloat16
    qm = np.asarray(QMask).astype(bool)
    km = np.asarray(KMask).astype(bool)
    qidxs = [np.nonzero(qm[b])[0] for b in range(B)]
    kidxs = [np.nonzero(km[b])[0] for b in range(B)]
    ntq = max(1, math.ceil(max(len(ix) for ix in qidxs) / P))
    ntk = max(1, math.ceil(max(len(ix) for ix in kidxs) / P))
    Lq, Lk = ntq * P, ntk * P

    w2 = np.zeros((P, NCH, P), dtype=np.float32)
    hl = np.asarray(HeadLinear, dtype=np.float32)
    for c in range(NCH):
        w2[0:64, c, 0:64] = hl[2 * c]
        w2[64:128, c, 64:128] = hl[2 * c + 1]
    w2b = w2.astype(bf16)
    ob = np.asarray(OutputLiner, dtype=np.float32).astype(bf16)

    in_maps = []
    for b in range(B):
        qi, ki = qidxs[b], kidxs[b]
        qc = np.zeros((Lq, E), dtype=np.float32)
        qc[:len(qi)] = np.asarray(Q[b], dtype=np.float32)[qi]
        kc = np.zeros((Lk, E), dtype=np.float32)
        kc[:len(ki)] = np.asarray(K[b], dtype=np.float32)[ki]
        vc = np.zeros((Lk, E), dtype=np.float32)
        vc[:len(ki)] = np.asarray(V[b], dtype=np.float32)[ki]
        kmc = np.zeros(Lk, dtype=np.float32)
        kmc[:len(ki)] = 1.0
        in_maps.append({
            "QT": np.ascontiguousarray(qc.T.astype(bf16)),
            "KT": np.ascontiguousarray(kc.T.astype(bf16)),
            "VT": np.ascontiguousarray(vc.T.astype(bf16)),
            "W2": w2b, "OB": ob,
            "KM": np.ascontiguousarray(kmc.reshape(ntk, P).T),
        })
    return in_maps, qidxs, ntq, ntk


_NC_CACHE = {}


def _get_nc(ntq, ntk):
    if (ntq, ntk) not in _NC_CACHE:
        _NC_CACHE[(ntq, ntk)] = build_bass(ntq, ntk)
    return _NC_CACHE[(ntq, ntk)]


def kernel(Q, K, V, HeadLinear, OutputLiner, QMask, KMask):
    from concourse.bass_utils import run_bass_kernel_spmd

    in_maps, qidxs, ntq, ntk = make_core_inputs(
        Q, K, V, HeadLinear, OutputLiner, QMask, KMask)
    nc = _get_nc(ntq, ntk)
    res = run_bass_kernel_spmd(nc, in_maps, list(range(B)))
    out = np.zeros((B, L, E), dtype=np.float32)
    for b in range(B):
        yc = np.asarray(res.results[b]["Y"])
        out[b][qidxs[b]] = yc[:len(qidxs[b])]
    return out
